# revision 6
# baseline (speedup 1.0000x reference)
"""Trainium2 kernel for nn_AEDecoder: out = LeakyReLU(X @ W_sparse + bias).

The sparse edge list (400k edges over a [1639, 17000] weight matrix, 1.4%
dense) is converted on the host to a dense bf16 weight matrix — the layout the
TensorEngine consumes — with the bias folded in as an extra ones-row of X.
Each of the 8 NeuronCores gets a 2125-gene column shard of W (data-parallel
over output genes, X replicated), runs a tiled bf16 matmul with f32 PSUM
accumulation and a fused LeakyReLU epilogue, and the host concatenates the
per-core outputs.

Device schedule (per core), v2 — tuned from the ntff trace of the v1 kernel
(68.2us: 1.5us pre-warmup idle, 5us cold-clock warmup at N=512, 49.5us
near-perfect MM stream, ~4us output drain, ~8us fixed NEFF postamble):
- Warmup is 16 cheap N=128 matmuls (~0.11us each cold) instead of 12 N=512
  ones (~0.6us each): the PE still trips the HAM activity window but the real
  stream starts ~4.5us earlier, as soon as the k=0 chunks land.
- Input stream is deadline-ordered and balanced across the 3 DMA queues with
  one 128KB piece per queue per k-step: sync carries x0 then the wa low
  halves, scalar the wa high halves, gpsimd (slower SWDGE start) wa0's high
  half then x1..x12. wb chunks follow round-robin on all three queues.
- Pass 1 computes genes 0..1023 k-outer (chunk k consumed as it lands);
  pass 2a computes genes 1024..2047 for batch rows 0..255 k-outer; then
  batch rows 256..383 m-contiguous, the transposed 77-gene tail, and batch
  rows 384..511 last — so PSUM banks complete staggered and only the final
  two banks' drain + 2 HWDGE output DMAs remain after the last matmul.
- LeakyReLU + PSUM->SBUF drain alternates between the ACT engine (fused
  Lrelu) and the DVE (0.01x + 0.99*relu(x)) so banks release at twice the
  single-engine rate; outputs are written bf16 and upcast on the host.
"""

import sys

import numpy as np

for _p in ("/opt/trn_rl_repo", "/root/.axon_site/_ro/trn_rl_repo"):
    if _p not in sys.path:
        sys.path.append(_p)

import ml_dtypes

B, IN_F, OUT_F = 512, 1639, 17000
NCORES = 8
SHARD = OUT_F // NCORES      # 2125 output genes per core
K_PAD = 1640                 # 1639 TF rows + 1 bias row (last chunk K=104)
KC = 13                      # contraction chunks (12 x 128 + 1 x 104)
NEG_SLOPE = 0.01
NTILE = 512                  # PSUM bank width in f32
NMAIN = (SHARD // NTILE) * NTILE   # 2048 genes in batch-major layout
NTAIL = SHARD - NMAIN              # 77 genes in gene-major (transposed) layout
MC = B // 128                # 4 batch chunks
WARMUP_MM = 18               # cheap N=128 matmuls to ramp the HAM clock gate

_cache: dict = {}


def _build_nc():
    import concourse.tile as tile
    from concourse import bacc, mybir
    nc = bacc.Bacc(
        "TRN2",
        target_bir_lowering=False,
        debug=False,
        num_devices=NCORES,
    )
    xT = nc.dram_tensor("xT", [K_PAD, B], mybir.dt.bfloat16, kind="ExternalInput").ap()
    w = nc.dram_tensor("w", [K_PAD, SHARD], mybir.dt.bfloat16, kind="ExternalInput").ap()
    out = nc.dram_tensor("out", [B, NMAIN], mybir.dt.bfloat16, kind="ExternalOutput").ap()
    out_t = nc.dram_tensor(
        "out_t", [NTAIL, B], mybir.dt.bfloat16, kind="ExternalOutput"
    ).ap()

    bf16 = mybir.dt.bfloat16
    f32 = mybir.dt.float32
    Lrelu = mybir.ActivationFunctionType.Lrelu

    with tile.TileContext(nc) as tc:
        with (
            tc.tile_pool(name="xp", bufs=1) as xp,
            tc.tile_pool(name="wp", bufs=1) as wp,
            tc.tile_pool(name="sp", bufs=1) as sp,
            tc.tile_pool(name="pp", bufs=8, space="PSUM") as pp,
            tc.tile_pool(name="op", bufs=6) as op,
        ):
            # Cheap warmup: N=128 matmuls keep the PE busy from t=0 so the
            # HAM clock gate ramps toward 2.4 GHz while the first input
            # chunks are still in flight.
            scr_t = sp.tile([128, 128], bf16, tag="scr")
            nc.gpsimd.memset(scr_t[:], 0.0)
            scr = scr_t[:]
            scr_ps = pp.tile([128, 128], f32, tag="psum", name="scr_ps")
            for _ in range(WARMUP_MM):
                nc.tensor.matmul(scr_ps[:], lhsT=scr, rhs=scr, start=True, stop=True)

            # Deadline-ordered input stream: per k-step one 128KB piece per
            # queue (x_k, wa_k lo half, wa_k hi half). sync/scalar are HWDGE
            # (fast first byte) and carry the k=0 critical pieces; gpsimd's
            # SWDGE pays ~1us setup so it gets wa0's hi half (first needed
            # by MM #5) and the x_k tail.
            xts, was, wbs = [], [], []
            for k in range(KC):
                kr = min(128, K_PAD - k * 128)
                xts.append(xp.tile([kr, B], bf16, tag=f"x{k}", name=f"x_{k}"))
                was.append(wp.tile([kr, 2 * NTILE], bf16, tag=f"wa{k}", name=f"wa_{k}"))
            nc.sync.dma_start(xts[0][:], xT[0:128, :])
            nc.scalar.dma_start(was[0][:, 0:NTILE], w[0:128, 0:NTILE])
            nc.sync.dma_start(was[0][:, NTILE : 2 * NTILE], w[0:128, NTILE : 2 * NTILE])
            for k in range(1, KC):
                kr = min(128, K_PAD - k * 128)
                ks = k * 128
                nc.gpsimd.dma_start(xts[k][:], xT[ks : ks + kr, :])
                nc.sync.dma_start(was[k][:, 0:NTILE], w[ks : ks + kr, 0:NTILE])
                nc.scalar.dma_start(
                    was[k][:, NTILE : 2 * NTILE], w[ks : ks + kr, NTILE : 2 * NTILE]
                )
            NB2 = SHARD - 2 * NTILE
            wb_eng = [nc.gpsimd, nc.scalar, nc.sync]
            for k in range(KC):
                kr = min(128, K_PAD - k * 128)
                wb = wp.tile([kr, NB2], bf16, tag=f"wb{k}", name=f"wb_{k}")
                wb_eng[k % 3].dma_start(
                    wb[:], w[k * 128 : k * 128 + kr, 2 * NTILE : SHARD]
                )
                wbs.append(wb)

            def w_slice(k, n):
                if n < 2:
                    return was[k][:, n * NTILE : (n + 1) * NTILE]
                return wbs[k][:, (n - 2) * NTILE : (n - 1) * NTILE]

            out_engs = [nc.gpsimd, nc.sync, nc.scalar]
            drain_i = [0]

            def drain(ps, n, m, eng_override=None):
                """LeakyReLU PSUM->SBUF (alternating ACT/DVE) + out DMA."""
                i = drain_i[0]
                drain_i[0] += 1
                ot = op.tile([128, NTILE], bf16, tag="o", name=f"o_{n}_{m}")
                if i % 2 == 0:
                    nc.scalar.activation(ot[:], ps[:], Lrelu, alpha=NEG_SLOPE)
                else:
                    # lrelu(x) = 0.01x + 0.99*relu(x); PSUM may only be read
                    # once per DVE instruction, so stage relu in SBUF.
                    rt = op.tile([128, NTILE], bf16, tag="r", name=f"r_{n}_{m}")
                    nc.vector.tensor_scalar(
                        rt[:], ps[:], 0.0, 1.0 - NEG_SLOPE,
                        mybir.AluOpType.max, mybir.AluOpType.mult,
                    )
                    nc.vector.scalar_tensor_tensor(
                        ot[:], ps[:], NEG_SLOPE, rt[:],
                        mybir.AluOpType.mult, mybir.AluOpType.add,
                    )
                eng = eng_override if eng_override is not None else out_engs[i % 3]
                eng.dma_start(
                    out[m * 128 : (m + 1) * 128, n * NTILE : (n + 1) * NTILE], ot[:]
                )

            def k_outer_block(ns, ms):
                """k-outer accumulation for the (n, m) tile set; paced with
                the chunk stream. Returns the psum tiles keyed by (n, m)."""
                pts = {
                    (n, m): pp.tile([128, NTILE], f32, tag="psum", name=f"ps_{n}_{m}")
                    for n in ns
                    for m in ms
                }
                for k in range(KC):
                    for n in ns:
                        for m in ms:
                            nc.tensor.matmul(
                                pts[(n, m)][:],
                                lhsT=xts[k][:, m * 128 : (m + 1) * 128],
                                rhs=w_slice(k, n),
                                start=(k == 0),
                                stop=(k == KC - 1),
                            )
                return pts

            def nm_sweep(n, m, eng):
                """Single-bank k-contiguous sweep; drained on the ACT engine
                and written via an HWDGE queue so banks retire mid-stream."""
                ps = pp.tile([128, NTILE], f32, tag="psum", name=f"ps_{n}_{m}")
                for k in range(KC):
                    nc.tensor.matmul(
                        ps[:],
                        lhsT=xts[k][:, m * 128 : (m + 1) * 128],
                        rhs=w_slice(k, n),
                        start=(k == 0),
                        stop=(k == KC - 1),
                    )
                ot = op.tile([128, NTILE], bf16, tag="o", name=f"o_{n}_{m}")
                nc.scalar.activation(ot[:], ps[:], Lrelu, alpha=NEG_SLOPE)
                eng.dma_start(
                    out[m * 128 : (m + 1) * 128, n * NTILE : (n + 1) * NTILE], ot[:]
                )

            # Pass 1: genes 0..1023, all batch rows, k-outer (DMA-paced).
            p1 = k_outer_block((0, 1), range(MC))
            for n in (0, 1):
                for m in range(MC):
                    drain(p1[(n, m)], n, m)
            # Pass 2a: genes 1024..2047, batch rows 0..255, k-outer (paces
            # the wb chunk stream which is still landing).
            p2 = k_outer_block((2, 3), (0, 1))
            for n in (2, 3):
                for m in (0, 1):
                    drain(p2[(n, m)], n, m)
            # Batch rows 256..511: single-bank k-contiguous sweeps so the
            # banks complete staggered and each drain+DMA overlaps the next
            # sweep's matmuls.
            nm_sweep(2, 2, nc.sync)
            nm_sweep(3, 2, nc.scalar)
            nm_sweep(2, 3, nc.sync)
            nm_sweep(3, 3, nc.scalar)

            # Gene tail last, transposed: 13 full-width matmuls cover the
            # final 77 genes; after the last matmul only this one ACT drain
            # and a single contiguous 79KB HWDGE write remain.
            tail_ps = pp.tile([NTAIL, B], f32, tag="psum", name="tail_ps")
            for k in range(KC):
                nc.tensor.matmul(
                    tail_ps[:],
                    lhsT=wbs[k][:, NMAIN - 2 * NTILE : NB2],
                    rhs=xts[k][:],
                    start=(k == 0),
                    stop=(k == KC - 1),
                )
            ot_t = op.tile([NTAIL, B], bf16, tag="ot")
            nc.scalar.activation(ot_t[:], tail_ps[:], Lrelu, alpha=NEG_SLOPE)
            nc.sync.dma_start(out_t[:, :], ot_t[:])

    nc.compile()
    return nc


def _prep_inputs(features, weights, bias, edge_out, edge_in):
    features = np.asarray(features, dtype=np.float32)
    weights = np.asarray(weights, dtype=np.float32)
    bias = np.asarray(bias, dtype=np.float32)
    ei = np.asarray(edge_in).astype(np.int64)
    eo = np.asarray(edge_out).astype(np.int64)

    # Sparse edge list -> dense [K_PAD, OUT_F] weight matrix, bias as row IN_F.
    W = np.zeros((K_PAD, OUT_F), dtype=np.float32)
    np.add.at(W, (ei, eo), weights)
    W[IN_F, :] = bias

    xT = np.zeros((K_PAD, B), dtype=np.float32)
    xT[:IN_F] = features.T
    xT[IN_F] = 1.0

    Wb = W.astype(ml_dtypes.bfloat16)
    xTb = np.ascontiguousarray(xT.astype(ml_dtypes.bfloat16))
    return [
        {
            "xT": xTb,
            "w": np.ascontiguousarray(Wb[:, c * SHARD : (c + 1) * SHARD]),
        }
        for c in range(NCORES)
    ]


def _assemble(results):
    cols = []
    for c in range(NCORES):
        cols.append(results[c]["out"].astype(np.float32))
        cols.append(results[c]["out_t"].T.astype(np.float32))
    return np.concatenate(cols, axis=1)


def kernel(features, weights, bias, edge_out, edge_in):
    from concourse import bass_utils

    in_maps = _prep_inputs(features, weights, bias, edge_out, edge_in)
    if "nc" not in _cache:
        _cache["nc"] = _build_nc()
    nc = _cache["nc"]
    res = bass_utils.run_bass_kernel_spmd(nc, in_maps, core_ids=list(range(NCORES)))
    return _assemble(res.results)


# revision 10
# speedup vs baseline: 1.0535x; 1.0535x over previous
"""Trainium2 kernel for nn_AEDecoder: out = LeakyReLU(X @ W_sparse + bias).

The sparse edge list (400k edges over a [1639, 17000] weight matrix, 1.4%
dense) is converted on the host to a dense bf16 weight matrix — the layout the
TensorEngine consumes — with the bias folded in as an extra ones-row of X.
Each of the 8 NeuronCores gets a 2125-gene column shard of W (data-parallel
over output genes, X replicated), runs a tiled bf16 matmul with f32 PSUM
accumulation and a fused LeakyReLU epilogue, and the host concatenates the
per-core outputs.

Device schedule (per core), v2 — tuned from the ntff trace of the v1 kernel
(68.2us: 1.5us pre-warmup idle, 5us cold-clock warmup at N=512, 49.5us
near-perfect MM stream, ~4us output drain, ~8us fixed NEFF postamble):
- Warmup is 16 cheap N=128 matmuls (~0.11us each cold) instead of 12 N=512
  ones (~0.6us each): the PE still trips the HAM activity window but the real
  stream starts ~4.5us earlier, as soon as the k=0 chunks land.
- Input stream is deadline-ordered and balanced across the 3 DMA queues with
  one 128KB piece per queue per k-step: sync carries x0 then the wa low
  halves, scalar the wa high halves, gpsimd (slower SWDGE start) wa0's high
  half then x1..x12. wb chunks follow round-robin on all three queues.
- Pass 1 computes genes 0..1023 k-outer (chunk k consumed as it lands);
  pass 2a computes genes 1024..2047 for batch rows 0..255 k-outer; then
  batch rows 256..383 m-contiguous, the transposed 77-gene tail, and batch
  rows 384..511 last — so PSUM banks complete staggered and only the final
  two banks' drain + 2 HWDGE output DMAs remain after the last matmul.
- LeakyReLU + PSUM->SBUF drain alternates between the ACT engine (fused
  Lrelu) and the DVE (0.01x + 0.99*relu(x)) so banks release at twice the
  single-engine rate; outputs are written bf16 and upcast on the host.
"""

import sys

import numpy as np

for _p in ("/opt/trn_rl_repo", "/root/.axon_site/_ro/trn_rl_repo"):
    if _p not in sys.path:
        sys.path.append(_p)

import ml_dtypes

B, IN_F, OUT_F = 512, 1639, 17000
NCORES = 8
SHARD = OUT_F // NCORES      # 2125 output genes per core
K_PAD = 1640                 # 1639 TF rows + 1 bias row (last chunk K=104)
KC = 13                      # contraction chunks (12 x 128 + 1 x 104)
NEG_SLOPE = 0.01
NTILE = 512                  # PSUM bank width in f32
NMAIN = (SHARD // NTILE) * NTILE   # 2048 genes in batch-major layout
NTAIL = SHARD - NMAIN              # 77 genes in gene-major (transposed) layout
MC = B // 128                # 4 batch chunks
WARMUP_MM = 18               # cheap N=128 matmuls to ramp the HAM clock gate

_cache: dict = {}


def _build_nc():
    import concourse.tile as tile
    from concourse import bacc, mybir
    nc = bacc.Bacc(
        "TRN2",
        target_bir_lowering=False,
        debug=False,
        num_devices=NCORES,
    )
    NB2 = SHARD - 2 * NTILE
    xT = nc.dram_tensor("xT", [K_PAD, B], mybir.dt.bfloat16, kind="ExternalInput").ap()
    # W arrives pre-split into the pass-1 and pass-2 column blocks so every
    # 128-row chunk DMA reads a fully contiguous DRAM range (2KB descriptor
    # lines instead of 1KB strided rows — ~2x per-queue DMA throughput).
    wA = nc.dram_tensor(
        "wa", [K_PAD, 2 * NTILE], mybir.dt.bfloat16, kind="ExternalInput"
    ).ap()
    wB = nc.dram_tensor(
        "wb", [K_PAD, NB2], mybir.dt.bfloat16, kind="ExternalInput"
    ).ap()
    out = nc.dram_tensor("out", [B, NMAIN], mybir.dt.bfloat16, kind="ExternalOutput").ap()
    out_t = nc.dram_tensor(
        "out_t", [NTAIL, B], mybir.dt.bfloat16, kind="ExternalOutput"
    ).ap()

    bf16 = mybir.dt.bfloat16
    f32 = mybir.dt.float32
    Lrelu = mybir.ActivationFunctionType.Lrelu

    with tile.TileContext(nc) as tc:
        with (
            tc.tile_pool(name="xp", bufs=1) as xp,
            tc.tile_pool(name="wp", bufs=1) as wp,
            tc.tile_pool(name="sp", bufs=1) as sp,
            tc.tile_pool(name="pp", bufs=8, space="PSUM") as pp,
            tc.tile_pool(name="op", bufs=6) as op,
        ):
            # Cheap warmup: N=128 matmuls keep the PE busy from t=0 so the
            # HAM clock gate ramps toward 2.4 GHz while the first input
            # chunks are still in flight.
            scr_t = sp.tile([128, 128], bf16, tag="scr")
            nc.vector.memset(scr_t[:], 0.0)
            scr = scr_t[:]
            scr_ps = pp.tile([128, 128], f32, tag="psum", name="scr_ps")
            for _ in range(WARMUP_MM):
                nc.tensor.matmul(scr_ps[:], lhsT=scr, rhs=scr, start=True, stop=True)

            # Deadline-ordered input stream, whole contiguous chunks: sync
            # carries x0 + the odd wa chunks, scalar the even wa chunks,
            # gpsimd (slow SWDGE start) the x tail. wb chunks follow
            # round-robin on all three queues.
            xts, was, wbs = [], [], []
            for k in range(KC):
                kr = min(128, K_PAD - k * 128)
                xts.append(xp.tile([kr, B], bf16, tag=f"x{k}", name=f"x_{k}"))
                was.append(wp.tile([kr, 2 * NTILE], bf16, tag=f"wa{k}", name=f"wa_{k}"))
            nc.sync.dma_start(xts[0][:], xT[0:128, :])
            nc.scalar.dma_start(was[0][:], wA[0:128, :])
            for k in range(1, KC):
                kr = min(128, K_PAD - k * 128)
                ks = k * 128
                nc.gpsimd.dma_start(xts[k][:], xT[ks : ks + kr, :])
                eng = nc.sync if k % 2 == 1 else nc.scalar
                eng.dma_start(was[k][:], wA[ks : ks + kr, :])
            wb_eng = [nc.gpsimd, nc.scalar, nc.sync]
            for k in range(KC):
                kr = min(128, K_PAD - k * 128)
                wb = wp.tile([kr, NB2], bf16, tag=f"wb{k}", name=f"wb_{k}")
                wb_eng[k % 3].dma_start(wb[:], wB[k * 128 : k * 128 + kr, :])
                wbs.append(wb)

            def w_slice(k, n):
                if n < 2:
                    return was[k][:, n * NTILE : (n + 1) * NTILE]
                return wbs[k][:, (n - 2) * NTILE : (n - 1) * NTILE]

            out_engs = [nc.gpsimd, nc.sync, nc.scalar]
            drain_i = [0]

            def drain(ps, n, m, eng_override=None):
                """LeakyReLU PSUM->SBUF (alternating ACT/DVE) + out DMA."""
                i = drain_i[0]
                drain_i[0] += 1
                ot = op.tile([128, NTILE], bf16, tag="o", name=f"o_{n}_{m}")
                if i % 2 == 0:
                    nc.scalar.activation(ot[:], ps[:], Lrelu, alpha=NEG_SLOPE)
                else:
                    # lrelu(x) = 0.01x + 0.99*relu(x); PSUM may only be read
                    # once per DVE instruction, so stage relu in SBUF.
                    rt = op.tile([128, NTILE], bf16, tag="r", name=f"r_{n}_{m}")
                    nc.vector.tensor_scalar(
                        rt[:], ps[:], 0.0, 1.0 - NEG_SLOPE,
                        mybir.AluOpType.max, mybir.AluOpType.mult,
                    )
                    nc.vector.scalar_tensor_tensor(
                        ot[:], ps[:], NEG_SLOPE, rt[:],
                        mybir.AluOpType.mult, mybir.AluOpType.add,
                    )
                eng = eng_override if eng_override is not None else out_engs[i % 3]
                eng.dma_start(
                    out[m * 128 : (m + 1) * 128, n * NTILE : (n + 1) * NTILE], ot[:]
                )

            def k_outer_block(ns, ms):
                """k-outer accumulation for the (n, m) tile set; paced with
                the chunk stream. Returns the psum tiles keyed by (n, m)."""
                pts = {
                    (n, m): pp.tile([128, NTILE], f32, tag="psum", name=f"ps_{n}_{m}")
                    for n in ns
                    for m in ms
                }
                for k in range(KC):
                    for n in ns:
                        for m in ms:
                            nc.tensor.matmul(
                                pts[(n, m)][:],
                                lhsT=xts[k][:, m * 128 : (m + 1) * 128],
                                rhs=w_slice(k, n),
                                start=(k == 0),
                                stop=(k == KC - 1),
                            )
                return pts

            def nm_sweep(n, m, eng):
                """Single-bank k-contiguous sweep; drained on the ACT engine
                and written via an HWDGE queue so banks retire mid-stream."""
                ps = pp.tile([128, NTILE], f32, tag="psum", name=f"ps_{n}_{m}")
                for k in range(KC):
                    nc.tensor.matmul(
                        ps[:],
                        lhsT=xts[k][:, m * 128 : (m + 1) * 128],
                        rhs=w_slice(k, n),
                        start=(k == 0),
                        stop=(k == KC - 1),
                    )
                ot = op.tile([128, NTILE], bf16, tag="o", name=f"o_{n}_{m}")
                nc.scalar.activation(ot[:], ps[:], Lrelu, alpha=NEG_SLOPE)
                eng.dma_start(
                    out[m * 128 : (m + 1) * 128, n * NTILE : (n + 1) * NTILE], ot[:]
                )

            # Pass 1: genes 0..1023, all batch rows, k-outer (DMA-paced).
            p1 = k_outer_block((0, 1), range(MC))
            for n in (0, 1):
                for m in range(MC):
                    drain(p1[(n, m)], n, m)
            # Pass 2a: genes 1024..2047, batch rows 0..255, k-outer (paces
            # the wb chunk stream which is still landing).
            p2 = k_outer_block((2, 3), (0, 1))
            for n in (2, 3):
                for m in (0, 1):
                    drain(p2[(n, m)], n, m)
            # Batch rows 256..511: single-bank k-contiguous sweeps so the
            # banks complete staggered and each drain+DMA overlaps the next
            # sweep's matmuls.
            nm_sweep(2, 2, nc.sync)
            nm_sweep(3, 2, nc.scalar)
            nm_sweep(2, 3, nc.sync)
            nm_sweep(3, 3, nc.scalar)

            # Gene tail last, transposed: 13 full-width matmuls cover the
            # final 77 genes; after the last matmul only this one ACT drain
            # and a single contiguous 79KB HWDGE write remain.
            tail_ps = pp.tile([NTAIL, B], f32, tag="psum", name="tail_ps")
            for k in range(KC):
                nc.tensor.matmul(
                    tail_ps[:],
                    lhsT=wbs[k][:, NMAIN - 2 * NTILE : NB2],
                    rhs=xts[k][:],
                    start=(k == 0),
                    stop=(k == KC - 1),
                )
            ot_t = op.tile([NTAIL, B], bf16, tag="ot")
            nc.scalar.activation(ot_t[:], tail_ps[:], Lrelu, alpha=NEG_SLOPE)
            nc.sync.dma_start(out_t[:, :], ot_t[:])

    nc.compile()
    return nc


def _prep_inputs(features, weights, bias, edge_out, edge_in):
    features = np.asarray(features, dtype=np.float32)
    weights = np.asarray(weights, dtype=np.float32)
    bias = np.asarray(bias, dtype=np.float32)
    ei = np.asarray(edge_in).astype(np.int64)
    eo = np.asarray(edge_out).astype(np.int64)

    # Sparse edge list -> dense [K_PAD, OUT_F] weight matrix, bias as row IN_F.
    W = np.zeros((K_PAD, OUT_F), dtype=np.float32)
    np.add.at(W, (ei, eo), weights)
    W[IN_F, :] = bias

    xT = np.zeros((K_PAD, B), dtype=np.float32)
    xT[:IN_F] = features.T
    xT[IN_F] = 1.0

    Wb = W.astype(ml_dtypes.bfloat16)
    xTb = np.ascontiguousarray(xT.astype(ml_dtypes.bfloat16))
    NA = 2 * NTILE
    return [
        {
            "xT": xTb,
            "wa": np.ascontiguousarray(Wb[:, c * SHARD : c * SHARD + NA]),
            "wb": np.ascontiguousarray(Wb[:, c * SHARD + NA : (c + 1) * SHARD]),
        }
        for c in range(NCORES)
    ]


def _assemble(results):
    cols = []
    for c in range(NCORES):
        cols.append(results[c]["out"].astype(np.float32))
        cols.append(results[c]["out_t"].T.astype(np.float32))
    return np.concatenate(cols, axis=1)


def kernel(features, weights, bias, edge_out, edge_in):
    from concourse import bass_utils

    in_maps = _prep_inputs(features, weights, bias, edge_out, edge_in)
    if "nc" not in _cache:
        _cache["nc"] = _build_nc()
    nc = _cache["nc"]
    res = bass_utils.run_bass_kernel_spmd(nc, in_maps, core_ids=list(range(NCORES)))
    return _assemble(res.results)


# revision 12
# speedup vs baseline: 1.0700x; 1.0157x over previous
"""Trainium2 kernel for nn_AEDecoder: out = LeakyReLU(X @ W_sparse + bias).

The sparse edge list (400k edges over a [1639, 17000] weight matrix, 1.4%
dense) is converted on the host to a dense bf16 weight matrix — the layout the
TensorEngine consumes — with the bias folded in as an extra ones-row of X.
Each of the 8 NeuronCores gets a 2125-gene column shard of W (data-parallel
over output genes, X replicated), runs a tiled bf16 matmul with f32 PSUM
accumulation and a fused LeakyReLU epilogue, and the host concatenates the
per-core outputs.

Device schedule (per core), v2 — tuned from the ntff trace of the v1 kernel
(68.2us: 1.5us pre-warmup idle, 5us cold-clock warmup at N=512, 49.5us
near-perfect MM stream, ~4us output drain, ~8us fixed NEFF postamble):
- Warmup is 16 cheap N=128 matmuls (~0.11us each cold) instead of 12 N=512
  ones (~0.6us each): the PE still trips the HAM activity window but the real
  stream starts ~4.5us earlier, as soon as the k=0 chunks land.
- Input stream is deadline-ordered and balanced across the 3 DMA queues with
  one 128KB piece per queue per k-step: sync carries x0 then the wa low
  halves, scalar the wa high halves, gpsimd (slower SWDGE start) wa0's high
  half then x1..x12. wb chunks follow round-robin on all three queues.
- Pass 1 computes genes 0..1023 k-outer (chunk k consumed as it lands);
  pass 2a computes genes 1024..2047 for batch rows 0..255 k-outer; then
  batch rows 256..383 m-contiguous, the transposed 77-gene tail, and batch
  rows 384..511 last — so PSUM banks complete staggered and only the final
  two banks' drain + 2 HWDGE output DMAs remain after the last matmul.
- LeakyReLU + PSUM->SBUF drain alternates between the ACT engine (fused
  Lrelu) and the DVE (0.01x + 0.99*relu(x)) so banks release at twice the
  single-engine rate; outputs are written bf16 and upcast on the host.
"""

import sys

import numpy as np

for _p in ("/opt/trn_rl_repo", "/root/.axon_site/_ro/trn_rl_repo"):
    if _p not in sys.path:
        sys.path.append(_p)

import ml_dtypes

B, IN_F, OUT_F = 512, 1639, 17000
NCORES = 8
SHARD = OUT_F // NCORES      # 2125 output genes per core
K_PAD = 1640                 # 1639 TF rows + 1 bias row (last chunk K=104)
KC = 13                      # contraction chunks (12 x 128 + 1 x 104)
NEG_SLOPE = 0.01
NTILE = 512                  # PSUM bank width in f32
NMAIN = (SHARD // NTILE) * NTILE   # 2048 genes in batch-major layout
NTAIL = SHARD - NMAIN              # 77 genes in gene-major (transposed) layout
MC = B // 128                # 4 batch chunks
WARMUP_MM = 30               # cheap N=128 matmuls to ramp the HAM clock gate

_cache: dict = {}


def _build_nc():
    import concourse.tile as tile
    from concourse import bacc, mybir
    nc = bacc.Bacc(
        "TRN2",
        target_bir_lowering=False,
        debug=False,
        num_devices=NCORES,
    )
    NB2 = SHARD - 2 * NTILE
    xT = nc.dram_tensor("xT", [K_PAD, B], mybir.dt.bfloat16, kind="ExternalInput").ap()
    # W arrives pre-split into the pass-1 and pass-2 column blocks so every
    # 128-row chunk DMA reads a fully contiguous DRAM range (2KB descriptor
    # lines instead of 1KB strided rows — ~2x per-queue DMA throughput).
    wA = nc.dram_tensor(
        "wa", [K_PAD, 2 * NTILE], mybir.dt.bfloat16, kind="ExternalInput"
    ).ap()
    wB = nc.dram_tensor(
        "wb", [K_PAD, NB2], mybir.dt.bfloat16, kind="ExternalInput"
    ).ap()
    out = nc.dram_tensor("out", [B, NMAIN], mybir.dt.bfloat16, kind="ExternalOutput").ap()
    out_t = nc.dram_tensor(
        "out_t", [NTAIL, B], mybir.dt.bfloat16, kind="ExternalOutput"
    ).ap()

    bf16 = mybir.dt.bfloat16
    f32 = mybir.dt.float32
    Lrelu = mybir.ActivationFunctionType.Lrelu

    with tile.TileContext(nc) as tc:
        with (
            tc.tile_pool(name="xp", bufs=1) as xp,
            tc.tile_pool(name="wp", bufs=1) as wp,
            tc.tile_pool(name="sp", bufs=1) as sp,
            tc.tile_pool(name="pp", bufs=8, space="PSUM") as pp,
            tc.tile_pool(name="op", bufs=6) as op,
        ):
            # Cheap warmup: N=128 matmuls keep the PE busy from t=0 so the
            # HAM clock gate ramps toward 2.4 GHz while the first input
            # chunks are still in flight.
            scr_t = sp.tile([128, 128], bf16, tag="scr")
            nc.vector.memset(scr_t[:], 0.0)
            scr = scr_t[:]
            scr_ps = pp.tile([128, 128], f32, tag="psum", name="scr_ps")
            for _ in range(WARMUP_MM):
                nc.tensor.matmul(scr_ps[:], lhsT=scr, rhs=scr, start=True, stop=True)

            # Deadline-ordered input stream, whole contiguous chunks: sync
            # carries x0 + the odd wa chunks, scalar the even wa chunks,
            # gpsimd (slow SWDGE start) the x tail. wb chunks follow
            # round-robin on all three queues.
            xts, was, wbs = [], [], []
            for k in range(KC):
                kr = min(128, K_PAD - k * 128)
                xts.append(xp.tile([kr, B], bf16, tag=f"x{k}", name=f"x_{k}"))
                was.append(wp.tile([kr, 2 * NTILE], bf16, tag=f"wa{k}", name=f"wa_{k}"))
            nc.sync.dma_start(xts[0][:], xT[0:128, :])
            nc.scalar.dma_start(was[0][:], wA[0:128, :])
            for k in range(1, KC):
                kr = min(128, K_PAD - k * 128)
                ks = k * 128
                nc.gpsimd.dma_start(xts[k][:], xT[ks : ks + kr, :])
                eng = nc.sync if k % 2 == 1 else nc.scalar
                eng.dma_start(was[k][:], wA[ks : ks + kr, :])
            wb_eng = [nc.gpsimd, nc.scalar, nc.sync]
            for k in range(KC):
                kr = min(128, K_PAD - k * 128)
                wb = wp.tile([kr, NB2], bf16, tag=f"wb{k}", name=f"wb_{k}")
                wb_eng[k % 3].dma_start(wb[:], wB[k * 128 : k * 128 + kr, :])
                wbs.append(wb)

            def w_slice(k, n):
                if n < 2:
                    return was[k][:, n * NTILE : (n + 1) * NTILE]
                return wbs[k][:, (n - 2) * NTILE : (n - 1) * NTILE]

            out_engs = [nc.gpsimd, nc.sync, nc.scalar]
            drain_i = [0]

            def drain(ps, n, m, eng_override=None):
                """LeakyReLU PSUM->SBUF (alternating ACT/DVE) + out DMA."""
                i = drain_i[0]
                drain_i[0] += 1
                ot = op.tile([128, NTILE], bf16, tag="o", name=f"o_{n}_{m}")
                if i % 2 == 0:
                    nc.scalar.activation(ot[:], ps[:], Lrelu, alpha=NEG_SLOPE)
                else:
                    # lrelu(x) = 0.01x + 0.99*relu(x); PSUM may only be read
                    # once per DVE instruction, so stage relu in SBUF.
                    rt = op.tile([128, NTILE], bf16, tag="r", name=f"r_{n}_{m}")
                    nc.vector.tensor_scalar(
                        rt[:], ps[:], 0.0, 1.0 - NEG_SLOPE,
                        mybir.AluOpType.max, mybir.AluOpType.mult,
                    )
                    nc.vector.scalar_tensor_tensor(
                        ot[:], ps[:], NEG_SLOPE, rt[:],
                        mybir.AluOpType.mult, mybir.AluOpType.add,
                    )
                eng = eng_override if eng_override is not None else out_engs[i % 3]
                eng.dma_start(
                    out[m * 128 : (m + 1) * 128, n * NTILE : (n + 1) * NTILE], ot[:]
                )

            def k_outer_block(ns, ms):
                """k-outer accumulation for the (n, m) tile set; paced with
                the chunk stream. Returns the psum tiles keyed by (n, m)."""
                pts = {
                    (n, m): pp.tile([128, NTILE], f32, tag="psum", name=f"ps_{n}_{m}")
                    for n in ns
                    for m in ms
                }
                for k in range(KC):
                    for n in ns:
                        for m in ms:
                            nc.tensor.matmul(
                                pts[(n, m)][:],
                                lhsT=xts[k][:, m * 128 : (m + 1) * 128],
                                rhs=w_slice(k, n),
                                start=(k == 0),
                                stop=(k == KC - 1),
                            )
                return pts

            def nm_sweep(n, m, eng):
                """Single-bank k-contiguous sweep; drained on the ACT engine
                and written via an HWDGE queue so banks retire mid-stream."""
                ps = pp.tile([128, NTILE], f32, tag="psum", name=f"ps_{n}_{m}")
                for k in range(KC):
                    nc.tensor.matmul(
                        ps[:],
                        lhsT=xts[k][:, m * 128 : (m + 1) * 128],
                        rhs=w_slice(k, n),
                        start=(k == 0),
                        stop=(k == KC - 1),
                    )
                ot = op.tile([128, NTILE], bf16, tag="o", name=f"o_{n}_{m}")
                nc.scalar.activation(ot[:], ps[:], Lrelu, alpha=NEG_SLOPE)
                eng.dma_start(
                    out[m * 128 : (m + 1) * 128, n * NTILE : (n + 1) * NTILE], ot[:]
                )

            # Pass 1: genes 0..1023, all batch rows, k-outer (DMA-paced).
            p1 = k_outer_block((0, 1), range(MC))
            for n in (0, 1):
                for m in range(MC):
                    drain(p1[(n, m)], n, m)
            # Pass 2a: genes 1024..2047, batch rows 0..255, k-outer (paces
            # the wb chunk stream which is still landing).
            p2 = k_outer_block((2, 3), (0, 1))
            for n in (2, 3):
                for m in (0, 1):
                    drain(p2[(n, m)], n, m)
            # Batch rows 256..511: single-bank k-contiguous sweeps so the
            # banks complete staggered and each drain+DMA overlaps the next
            # sweep's matmuls.
            nm_sweep(2, 2, nc.sync)
            nm_sweep(3, 2, nc.scalar)
            nm_sweep(2, 3, nc.sync)
            nm_sweep(3, 3, nc.scalar)

            # Gene tail last, transposed: 13 full-width matmuls cover the
            # final 77 genes; after the last matmul only this one ACT drain
            # and a single contiguous 79KB HWDGE write remain.
            tail_ps = pp.tile([NTAIL, B], f32, tag="psum", name="tail_ps")
            for k in range(KC):
                nc.tensor.matmul(
                    tail_ps[:],
                    lhsT=wbs[k][:, NMAIN - 2 * NTILE : NB2],
                    rhs=xts[k][:],
                    start=(k == 0),
                    stop=(k == KC - 1),
                )
            # Drain the tail in column halves so the first HWDGE write is in
            # flight while the second half's LeakyReLU still runs.
            ot_t = op.tile([NTAIL, B], bf16, tag="ot")
            H = B // 2
            nc.scalar.activation(ot_t[:, 0:H], tail_ps[:, 0:H], Lrelu, alpha=NEG_SLOPE)
            nc.sync.dma_start(out_t[:, 0:H], ot_t[:, 0:H])
            nc.scalar.activation(ot_t[:, H:B], tail_ps[:, H:B], Lrelu, alpha=NEG_SLOPE)
            nc.scalar.dma_start(out_t[:, H:B], ot_t[:, H:B])

    nc.compile()
    return nc


def _prep_inputs(features, weights, bias, edge_out, edge_in):
    features = np.asarray(features, dtype=np.float32)
    weights = np.asarray(weights, dtype=np.float32)
    bias = np.asarray(bias, dtype=np.float32)
    ei = np.asarray(edge_in).astype(np.int64)
    eo = np.asarray(edge_out).astype(np.int64)

    # Sparse edge list -> dense [K_PAD, OUT_F] weight matrix, bias as row IN_F.
    W = np.zeros((K_PAD, OUT_F), dtype=np.float32)
    np.add.at(W, (ei, eo), weights)
    W[IN_F, :] = bias

    xT = np.zeros((K_PAD, B), dtype=np.float32)
    xT[:IN_F] = features.T
    xT[IN_F] = 1.0

    Wb = W.astype(ml_dtypes.bfloat16)
    xTb = np.ascontiguousarray(xT.astype(ml_dtypes.bfloat16))
    NA = 2 * NTILE
    return [
        {
            "xT": xTb,
            "wa": np.ascontiguousarray(Wb[:, c * SHARD : c * SHARD + NA]),
            "wb": np.ascontiguousarray(Wb[:, c * SHARD + NA : (c + 1) * SHARD]),
        }
        for c in range(NCORES)
    ]


def _assemble(results):
    cols = []
    for c in range(NCORES):
        cols.append(results[c]["out"].astype(np.float32))
        cols.append(results[c]["out_t"].T.astype(np.float32))
    return np.concatenate(cols, axis=1)


def kernel(features, weights, bias, edge_out, edge_in):
    from concourse import bass_utils

    in_maps = _prep_inputs(features, weights, bias, edge_out, edge_in)
    if "nc" not in _cache:
        _cache["nc"] = _build_nc()
    nc = _cache["nc"]
    res = bass_utils.run_bass_kernel_spmd(nc, in_maps, core_ids=list(range(NCORES)))
    return _assemble(res.results)
